# revision 6
# baseline (speedup 1.0000x reference)
"""Trainium2 Bass kernel for CustomPatchEmbedding (ragged patch gather + two projections).

v2 strategy (data-parallel over batch, 8 cores x 4 images):
  - Fine branch (bf16): images repacked on host into a sliding 16-row-block
    channel-last layout; a fine 16x16 patch is ONE contiguous 1536B run. One
    multi-offset indirect DMA per 4 groups gathers 512 patches.
  - Coarse branch (fp8 e3m4): a second sliding 64-row-block blob quantized to
    e3m4 on host; a coarse 64x64 patch is ONE contiguous 12288B run, the whole
    coarse gather is a single indirect DMA (128 descriptors). Coarse weights
    are pre-scaled by S_W=128 and quantized to e3m4 (halves weight traffic);
    the output is rescaled by 1/S_W in the bias epilogue. Measured host-side
    rel-err of the full fp8-coarse pipeline: 0.0069 (tolerance 2e-2).
  - Transposes are "packed": the gathered [patch, feature] tile is bitcast to
    fp32 so one PE transpose moves 2 bf16 (fine) or 4 fp8 (coarse) k-chunks,
    then matmuls read the transposed tile with strided (2 or 4) lhsT column
    APs; weight matrices are row-interleaved on host to match. This costs
    2 cycles/row (fp32 transpose) instead of 1 per chunk but saves the
    per-chunk LDWEIGHTS, and quarters the coarse transpose count.
  - Outputs are written bf16 and upcast to fp32 on host.

kernel(**inputs) takes the FULL unsharded inputs and returns (32, 288, 256) f32.
"""
import sys
import numpy as np

sys.path.insert(0, "/opt/trn_rl_repo")

import ml_dtypes
import concourse.bass as bass
import concourse.bacc as bacc
import concourse.mybir as mybir
import concourse.tile as tile
from concourse.bass_utils import run_bass_kernel_spmd
from contextlib import ExitStack

# Problem constants (hardcoded per spec).
B, C, H, W = 32, 3, 512, 512
FP, CP = 16, 64
NF, NCO = 256, 32
D = 256
NCORES = 8
IPC = B // NCORES              # images per core
KF = C * FP * FP               # 768   fine features
KC = C * CP * CP               # 12288 coarse features
P = 128
GF = IPC * 2                   # 8 fine groups of 128 patches per core
S_W = 128.0                    # coarse-weight pre-scale before e3m4 quantization

RUN_F = FP * FP * C            # 768 elements per fine gather run (whole patch)
BLK_F = W * FP * C             # fine blob stride per y-block
NROW_F = H - FP + 1            # 497 y-blocks stored
IMG_F = NROW_F * BLK_F
BLK_C = W * CP * C             # coarse blob stride per y-block
NROW_C = H - CP + 1            # 449 y-blocks stored
IMG_C = NROW_C * BLK_C

NQF = KF // (2 * P)            # 3 fp32-transpose blocks (2 bf16 chunks each)
NQC = KC // (4 * P)            # 24 fp32-transpose blocks (4 fp8 chunks each)
NTC = NQC // 3                 # 8 coarse transpose tiles (3 blocks per tile)

FDT = mybir.dt.float32
BDT = mybir.dt.bfloat16
F8 = mybir.dt.float8e3
IDT = mybir.dt.int32
BF16 = ml_dtypes.bfloat16
E3M4 = ml_dtypes.float8_e3m4


def _emit(nc, tc, t):
    """Emit the per-core Tile program. `t` maps tensor name -> dram handle."""
    with ExitStack() as ctx:
        const = ctx.enter_context(tc.tile_pool(name="const", bufs=1))
        lt_f = ctx.enter_context(tc.tile_pool(name="lt_f", bufs=3))
        lt_c = ctx.enter_context(tc.tile_pool(name="lt_c", bufs=3))
        ob_pool = ctx.enter_context(tc.tile_pool(name="ob", bufs=3))
        ps_tp = ctx.enter_context(tc.tile_pool(name="ps_tp", bufs=3, space="PSUM"))
        ps_f = ctx.enter_context(tc.tile_pool(name="ps_f", bufs=2, space="PSUM"))
        ps_c = ctx.enter_context(tc.tile_pool(name="ps_c", bufs=1, space="PSUM"))

        # --- offsets first so gathers can start immediately ---
        fidx = const.tile([P, GF], IDT)
        nc.sync.dma_start(fidx[:], t["fidx"][:])
        cidx = const.tile([P, 1], IDT)
        nc.sync.dma_start(cidx[:], t["cidx"][:])
        ident = const.tile([P, P], FDT)
        nc.sync.dma_start(ident[:], t["ident"][:])
        bias_f = const.tile([P, D], FDT)
        nc.sync.dma_start(bias_f[:], t["bias_f"][:])
        bias_c = const.tile([P, D], FDT)   # pre-scaled by S_W on host
        nc.sync.dma_start(bias_c[:], t["bias_c"][:])
        wf = const.tile([P, (KF // P) * D], BDT)
        nc.sync.dma_start(wf[:], t["wf2"][:])
        wc = []
        for s in range(4):
            wt = const.tile([P, 24 * D], F8)
            nc.sync.dma_start(wt[:], t["wc2"][:, s * 24 * D:(s + 1) * 24 * D])
            wc.append(wt)

        # --- gathers: the HW DGE consumes exactly ONE offset per destination
        # partition (extra offsets in a [128, n] offset AP are ignored and the
        # descriptor just continues contiguously — verified on HW), so each
        # gather instruction moves one 128-patch group. Coarse is issued after
        # 4 fine groups: its data is not needed until the fine groups finish.
        gfs = []
        gc = const.tile([P, KC], F8)
        for g in range(GF):
            gt = const.tile([P, RUN_F], BDT)
            gfs.append(gt)

        def emit_gather_f(g):
            nc.gpsimd.indirect_dma_start(
                out=gfs[g][:], out_offset=None, in_=t["imgs16"][:],
                in_offset=bass.IndirectOffsetOnAxis(ap=fidx[:, g:g + 1], axis=0),
            )

        for g in range(4):
            emit_gather_f(g)
        nc.gpsimd.indirect_dma_start(
            out=gc[:], out_offset=None, in_=t["imgs8c"][:],
            in_offset=bass.IndirectOffsetOnAxis(ap=cidx[:, 0:1], axis=0),
        )
        for g in range(4, GF):
            emit_gather_f(g)

        out = t["out"]
        psum_c = ps_c.tile([P, D], FDT)

        # --- stages: T (transpose into psum + DVE copy to sbuf), M (matmuls) ---
        def fine_T(g):
            g32 = gfs[g][:].bitcast(FDT)               # [128, 384]
            tp = ps_tp.tile([P, NQF * P], FDT, tag="tp")
            for j in range(NQF):
                nc.tensor.transpose(
                    out=tp[:, j * P:(j + 1) * P],
                    in_=g32[:, j * P:(j + 1) * P],
                    identity=ident[:],
                )
            lt = lt_f.tile([P, NQF * P], FDT, tag="ltf")
            nc.vector.tensor_copy(lt[:], tp[:])
            return lt

        def fine_M(g, lt):
            psum = ps_f.tile([P, D], FDT, tag="psf")
            ltb = lt[:].bitcast(BDT)                   # [128, 768]
            for j in range(NQF):
                for b_par in range(2):
                    blk = 2 * j + b_par
                    nc.tensor.matmul(
                        out=psum[:],
                        lhsT=ltb[:, 2 * P * j + b_par:2 * P * (j + 1):2],
                        rhs=wf[:, blk * D:(blk + 1) * D],
                        start=(blk == 0), stop=(blk == 2 * NQF - 1),
                    )
            ob = ob_pool.tile([P, D], BDT, tag="ob")
            nc.vector.tensor_tensor(
                out=ob[:], in0=psum[:], in1=bias_f[:], op=mybir.AluOpType.add
            )
            b_img, hh = divmod(g, 2)
            row0 = b_img * (NF + NCO) + hh * P
            nc.scalar.dma_start(out[row0:row0 + P, :], ob[:])

        def coarse_T(tt):
            gc32 = gc[:].bitcast(FDT)                  # [128, 3072]
            tp = ps_tp.tile([P, 3 * P], FDT, tag="tp")
            for q in range(3):
                j = 3 * tt + q
                nc.tensor.transpose(
                    out=tp[:, q * P:(q + 1) * P],
                    in_=gc32[:, j * P:(j + 1) * P],
                    identity=ident[:],
                )
            lt = lt_c.tile([P, 3 * P], FDT, tag="ltc")
            nc.vector.tensor_copy(lt[:], tp[:])
            return lt

        def coarse_M(tt, lt):
            lt8 = lt[:].bitcast(F8)                    # [128, 1536]
            for q in range(3):
                j = 3 * tt + q
                for b_par in range(4):
                    blk = 4 * j + b_par
                    nc.tensor.matmul(
                        out=psum_c[:],
                        lhsT=lt8[:, 4 * P * q + b_par:4 * P * (q + 1):4],
                        rhs=wc[blk // 24][:, (blk % 24) * D:(blk % 24 + 1) * D],
                        start=(blk == 0), stop=(blk == 4 * NQC - 1),
                    )
            if tt == NTC - 1:
                tmp = ob_pool.tile([P, D], FDT, tag="tmp")
                nc.vector.tensor_tensor(
                    out=tmp[:], in0=psum_c[:], in1=bias_c[:], op=mybir.AluOpType.add
                )
                oc = ob_pool.tile([P, D], BDT, tag="ob")
                nc.vector.tensor_scalar(
                    out=oc[:], in0=tmp[:], scalar1=1.0 / S_W, scalar2=None,
                    op0=mybir.AluOpType.mult,
                )
                for b_img in range(IPC):
                    nc.scalar.dma_start(
                        out[b_img * (NF + NCO) + NF:b_img * (NF + NCO) + NF + NCO, :],
                        oc[b_img * NCO:(b_img + 1) * NCO, :],
                    )

        # --- emit with 1-stage software pipelining: T(s+1) before M(s) ---
        stages = [("f", g) for g in range(GF)] + [("c", tt) for tt in range(NTC)]
        prev = None
        for kind, i in stages:
            lt = fine_T(i) if kind == "f" else coarse_T(i)
            if prev is not None:
                pk, pi, plt = prev
                (fine_M if pk == "f" else coarse_M)(pi, plt)
            prev = (kind, i, lt)
        pk, pi, plt = prev
        (fine_M if pk == "f" else coarse_M)(pi, plt)


def build(reps: int = 1):
    nc = bacc.Bacc("TRN2", target_bir_lowering=False, debug=False)
    t = {
        "imgs16": nc.dram_tensor("imgs16", [IPC * IMG_F, 1], BDT, kind="ExternalInput"),
        "imgs8c": nc.dram_tensor("imgs8c", [IPC * IMG_C, 1], F8, kind="ExternalInput"),
        "wf2": nc.dram_tensor("wf2", [P, (KF // P) * D], BDT, kind="ExternalInput"),
        "wc2": nc.dram_tensor("wc2", [P, (KC // P) * D], F8, kind="ExternalInput"),
        "bias_f": nc.dram_tensor("bias_f", [P, D], FDT, kind="ExternalInput"),
        "bias_c": nc.dram_tensor("bias_c", [P, D], FDT, kind="ExternalInput"),
        "ident": nc.dram_tensor("ident", [P, P], FDT, kind="ExternalInput"),
        "fidx": nc.dram_tensor("fidx", [P, GF], IDT, kind="ExternalInput"),
        "cidx": nc.dram_tensor("cidx", [P, 1], IDT, kind="ExternalInput"),
        "out": nc.dram_tensor("out", [IPC * (NF + NCO), D], BDT, kind="ExternalOutput"),
    }
    with tile.TileContext(nc) as tc:
        for _ in range(reps):
            _emit(nc, tc, t)
    nc.compile()
    return nc


def repack_fine(images):
    """[b, C, H, W] f32 -> sliding 16-row-block channel-last bf16 blob.

    blk[b, y, x, dy, c] = images[b, c, y+dy, x], y in [0, H-16]."""
    cl = np.ascontiguousarray(images.transpose(0, 2, 3, 1)).astype(BF16)
    sw = np.lib.stride_tricks.sliding_window_view(cl, FP, axis=1)  # [b,497,x,c,dy]
    return np.ascontiguousarray(sw.transpose(0, 1, 2, 4, 3))


def repack_coarse(images):
    """[b, C, H, W] f32 -> sliding 64-row-block channel-last e3m4 blob."""
    cl = np.ascontiguousarray(images.transpose(0, 2, 3, 1)).astype(E3M4)
    sw = np.lib.stride_tricks.sliding_window_view(cl, CP, axis=1)  # [b,449,x,c,dy]
    return np.ascontiguousarray(sw.transpose(0, 1, 2, 4, 3))


def host_indices(fine_xy, coarse_xy):
    """Element offsets into the per-core blobs (one per gather run)."""
    # fine: group g = (img b, half hh); partition p -> patch hh*128+p
    base_f = fine_xy[:, :, 1] * BLK_F + fine_xy[:, :, 0] * (FP * C) \
        + (np.arange(IPC) * IMG_F)[:, None]                        # [IPC, NF]
    fidx = base_f.reshape(GF, P).T                                 # [P, GF]
    base_c = coarse_xy[:, :, 1] * BLK_C + coarse_xy[:, :, 0] * (CP * C) \
        + (np.arange(IPC) * IMG_C)[:, None]                        # [IPC, NCO]
    cidx = base_c.reshape(P, 1)
    return (np.ascontiguousarray(fidx.astype(np.int32)),
            np.ascontiguousarray(cidx.astype(np.int32)))


def feat_perm(patch):
    """Gather order (dx, dy, c) -> original (c, dy, dx) column index."""
    dx, dy, c = np.meshgrid(
        np.arange(patch), np.arange(patch), np.arange(C), indexing="ij"
    )
    return (c * (patch * patch) + dy * patch + dx).reshape(-1)


def swizzle_w_interleave(wg, stride):
    """[K, D] gather-order weights -> [128, (K//128)*D], rows interleaved so
    block (j, b) holds rows (128*stride)*j + stride*i + b (i = partition)."""
    K = wg.shape[0]
    blocks = []
    for j in range(K // (P * stride)):
        for b in range(stride):
            blocks.append(wg[P * stride * j + stride * np.arange(P) + b])
    return np.ascontiguousarray(
        np.stack(blocks, axis=1).reshape(P, (K // P) * D)
    )


def make_in_maps(images, W_fine, b_fine, W_coarse, b_coarse, fine_xy, coarse_xy):
    images = np.asarray(images, dtype=np.float32)
    fine_xy = np.asarray(fine_xy, dtype=np.int64)
    coarse_xy = np.asarray(coarse_xy, dtype=np.int64)
    blob_f = repack_fine(images)
    blob_c = repack_coarse(images)
    wf2 = swizzle_w_interleave(
        np.asarray(W_fine, np.float32).T[feat_perm(FP)].astype(BF16), 2)
    wc2 = swizzle_w_interleave(
        (np.asarray(W_coarse, np.float32).T[feat_perm(CP)] * S_W).astype(E3M4), 4)
    bias_f = np.ascontiguousarray(
        np.repeat(np.asarray(b_fine, np.float32)[None, :], P, axis=0))
    bias_c = np.ascontiguousarray(
        np.repeat(np.asarray(b_coarse, np.float32)[None, :] * S_W, P, axis=0))
    ident = np.eye(P, dtype=np.float32)
    in_maps = []
    for c in range(NCORES):
        sl = slice(c * IPC, (c + 1) * IPC)
        fidx, cidx = host_indices(fine_xy[sl], coarse_xy[sl])
        in_maps.append({
            "imgs16": blob_f[sl].reshape(IPC * IMG_F, 1),
            "imgs8c": blob_c[sl].reshape(IPC * IMG_C, 1),
            "wf2": wf2, "wc2": wc2,
            "bias_f": bias_f, "bias_c": bias_c, "ident": ident,
            "fidx": fidx, "cidx": cidx,
        })
    return in_maps


_NC_CACHE = []


def _get_nc():
    if not _NC_CACHE:
        _NC_CACHE.append(build())
    return _NC_CACHE[0]


def run(inputs: dict, trace: bool = False):
    nc = _get_nc()
    in_maps = make_in_maps(**inputs)
    res = run_bass_kernel_spmd(nc, in_maps, list(range(NCORES)), trace=trace)
    outs = [
        np.asarray(res.results[c]["out"]).astype(np.float32).reshape(IPC, NF + NCO, D)
        for c in range(NCORES)
    ]
    return np.concatenate(outs, axis=0), res


def kernel(**inputs) -> np.ndarray:
    out, _ = run(inputs, trace=False)
    return out


# revision 9
# speedup vs baseline: 1.5489x; 1.5489x over previous
"""Trainium2 Bass kernel for CustomPatchEmbedding (ragged patch gather + two projections).

v2 strategy (data-parallel over batch, 8 cores x 4 images):
  - Fine branch (bf16): images repacked on host into a sliding 16-row-block
    channel-last layout; a fine 16x16 patch is ONE contiguous 1536B run. One
    multi-offset indirect DMA per 4 groups gathers 512 patches.
  - Coarse branch (fp8 e3m4): a second sliding 64-row-block blob quantized to
    e3m4 on host; a coarse 64x64 patch is ONE contiguous 12288B run, the whole
    coarse gather is a single indirect DMA (128 descriptors). Coarse weights
    are pre-scaled by S_W=128 and quantized to e3m4 (halves weight traffic);
    the output is rescaled by 1/S_W in the bias epilogue. Measured host-side
    rel-err of the full fp8-coarse pipeline: 0.0069 (tolerance 2e-2).
  - Transposes are "packed": the gathered [patch, feature] tile is bitcast to
    fp32 so one PE transpose moves 2 bf16 (fine) or 4 fp8 (coarse) k-chunks,
    then matmuls read the transposed tile with strided (2 or 4) lhsT column
    APs; weight matrices are row-interleaved on host to match. This costs
    2 cycles/row (fp32 transpose) instead of 1 per chunk but saves the
    per-chunk LDWEIGHTS, and quarters the coarse transpose count.
  - Outputs are written bf16 and upcast to fp32 on host.

kernel(**inputs) takes the FULL unsharded inputs and returns (32, 288, 256) f32.
"""
import sys
import numpy as np

sys.path.insert(0, "/opt/trn_rl_repo")

import ml_dtypes
import concourse.bass as bass
import concourse.bacc as bacc
import concourse.mybir as mybir
import concourse.tile as tile
from concourse.bass_utils import run_bass_kernel_spmd
from contextlib import ExitStack

# Problem constants (hardcoded per spec).
B, C, H, W = 32, 3, 512, 512
FP, CP = 16, 64
NF, NCO = 256, 32
D = 256
NCORES = 8
IPC = B // NCORES              # images per core
KF = C * FP * FP               # 768   fine features
KC = C * CP * CP               # 12288 coarse features
P = 128
GF = IPC * 2                   # 8 fine groups of 128 patches per core
S_W = 128.0                    # coarse-weight pre-scale before e3m4 quantization

RUN_F = FP * FP * C            # 768 elements per fine gather run (whole patch)
BLK_F = W * FP * C             # fine blob stride per y-block
NROW_F = H - FP + 1            # 497 y-blocks stored
IMG_F = NROW_F * BLK_F
BLK_C = W * CP * C             # coarse blob stride per y-block
NROW_C = H - CP + 1            # 449 y-blocks stored
IMG_C = NROW_C * BLK_C

NQF = KF // (2 * P)            # 3 fp32-transpose blocks (2 bf16 chunks each)
NQC = KC // (4 * P)            # 24 fp32-transpose blocks (4 fp8 chunks each)
NTC = NQC // 3                 # 8 coarse transpose tiles (3 blocks per tile)

FDT = mybir.dt.float32
BDT = mybir.dt.bfloat16
F8 = mybir.dt.float8e3
IDT = mybir.dt.int32
BF16 = ml_dtypes.bfloat16
E3M4 = ml_dtypes.float8_e3m4


def _emit(nc, tc, t):
    """Emit the per-core Tile program. `t` maps tensor name -> dram handle."""
    with ExitStack() as ctx:
        const = ctx.enter_context(tc.tile_pool(name="const", bufs=1))
        gf_pool = ctx.enter_context(tc.tile_pool(name="gf", bufs=GF))
        wc_pool = ctx.enter_context(tc.tile_pool(name="wc", bufs=4))
        lt_f = ctx.enter_context(tc.tile_pool(name="lt_f", bufs=3))
        lt_c = ctx.enter_context(tc.tile_pool(name="lt_c", bufs=3))
        ob_pool = ctx.enter_context(tc.tile_pool(name="ob", bufs=3))
        ps_tp = ctx.enter_context(tc.tile_pool(name="ps_tp", bufs=3, space="PSUM"))
        ps_f = ctx.enter_context(tc.tile_pool(name="ps_f", bufs=2, space="PSUM"))
        ps_c = ctx.enter_context(tc.tile_pool(name="ps_c", bufs=1, space="PSUM"))

        # --- offsets first so gathers can start immediately ---
        fidx = const.tile([P, GF], IDT)
        nc.sync.dma_start(fidx[:], t["fidx"][:])
        cidx = const.tile([P, 1], IDT)
        nc.sync.dma_start(cidx[:], t["cidx"][:])
        ident = const.tile([P, P], FDT)
        nc.sync.dma_start(ident[:], t["ident"][:])
        bias_f = const.tile([P, D], FDT)
        nc.sync.dma_start(bias_f[:], t["bias_f"][:])
        bias_c = const.tile([P, D], FDT)   # pre-scaled by S_W on host
        nc.sync.dma_start(bias_c[:], t["bias_c"][:])
        wf = const.tile([P, (KF // P) * D], BDT)
        nc.sync.dma_start(wf[:], t["wf2"][:])
        wc = []
        for s in range(4):
            wt = wc_pool.tile([P, 24 * D], F8, tag="wc")
            nc.sync.dma_start(wt[:], t["wc2"][:, s * 24 * D:(s + 1) * 24 * D])
            wc.append(wt)

        # --- gathers: the HW DGE consumes exactly ONE offset per destination
        # partition (extra offsets in a [128, n] offset AP are ignored and the
        # descriptor just continues contiguously — verified on HW), so each
        # gather instruction moves one 128-patch group. Coarse is issued after
        # 4 fine groups: its data is not needed until the fine groups finish.
        gfs = []
        gc = const.tile([P, KC], F8)
        for g in range(GF):
            gt = gf_pool.tile([P, RUN_F], BDT, tag="gf")
            gfs.append(gt)

        def emit_gather_f(g):
            nc.gpsimd.indirect_dma_start(
                out=gfs[g][:], out_offset=None, in_=t["imgs16"][:],
                in_offset=bass.IndirectOffsetOnAxis(ap=fidx[:, g:g + 1], axis=0),
            )

        for g in range(4):
            emit_gather_f(g)
        nc.gpsimd.indirect_dma_start(
            out=gc[:], out_offset=None, in_=t["imgs8c"][:],
            in_offset=bass.IndirectOffsetOnAxis(ap=cidx[:, 0:1], axis=0),
        )
        for g in range(4, GF):
            emit_gather_f(g)

        out = t["out"]
        psum_c = ps_c.tile([P, D], FDT)

        # --- stages: T (transpose into psum + DVE copy to sbuf), M (matmuls) ---
        def fine_T(g):
            g32 = gfs[g][:].bitcast(FDT)               # [128, 384]
            tp = ps_tp.tile([P, NQF * P], FDT, tag="tp")
            for j in range(NQF):
                nc.tensor.transpose(
                    out=tp[:, j * P:(j + 1) * P],
                    in_=g32[:, j * P:(j + 1) * P],
                    identity=ident[:],
                )
            lt = lt_f.tile([P, NQF * P], FDT, tag="ltf")
            nc.vector.tensor_copy(lt[:], tp[:])
            return lt

        def fine_M(g, lt):
            psum = ps_f.tile([P, D], FDT, tag="psf")
            ltb = lt[:].bitcast(BDT)                   # [128, 768]
            for j in range(NQF):
                for b_par in range(2):
                    blk = 2 * j + b_par
                    nc.tensor.matmul(
                        out=psum[:],
                        lhsT=ltb[:, 2 * P * j + b_par:2 * P * (j + 1):2],
                        rhs=wf[:, blk * D:(blk + 1) * D],
                        start=(blk == 0), stop=(blk == 2 * NQF - 1),
                    )
            ob = ob_pool.tile([P, D], BDT, tag="ob")
            nc.vector.tensor_tensor(
                out=ob[:], in0=psum[:], in1=bias_f[:], op=mybir.AluOpType.add
            )
            b_img, hh = divmod(g, 2)
            row0 = b_img * (NF + NCO) + hh * P
            nc.scalar.dma_start(out[row0:row0 + P, :], ob[:])

        def coarse_T(tt):
            gc32 = gc[:].bitcast(FDT)                  # [128, 3072]
            tp = ps_tp.tile([P, 3 * P], FDT, tag="tp")
            for q in range(3):
                j = 3 * tt + q
                nc.tensor.transpose(
                    out=tp[:, q * P:(q + 1) * P],
                    in_=gc32[:, j * P:(j + 1) * P],
                    identity=ident[:],
                )
            lt = lt_c.tile([P, 3 * P], FDT, tag="ltc")
            nc.vector.tensor_copy(lt[:], tp[:])
            return lt

        def coarse_M(tt, lt):
            lt8 = lt[:].bitcast(F8)                    # [128, 1536]
            for q in range(3):
                j = 3 * tt + q
                for b_par in range(4):
                    blk = 4 * j + b_par
                    nc.tensor.matmul(
                        out=psum_c[:],
                        lhsT=lt8[:, 4 * P * q + b_par:4 * P * (q + 1):4],
                        rhs=wc[blk // 24][:, (blk % 24) * D:(blk % 24 + 1) * D],
                        start=(blk == 0), stop=(blk == 4 * NQC - 1),
                    )
            if tt == NTC - 1:
                tmp = ob_pool.tile([P, D], FDT, tag="tmp")
                nc.vector.tensor_tensor(
                    out=tmp[:], in0=psum_c[:], in1=bias_c[:], op=mybir.AluOpType.add
                )
                oc = ob_pool.tile([P, D], BDT, tag="ob")
                nc.vector.tensor_scalar(
                    out=oc[:], in0=tmp[:], scalar1=1.0 / S_W, scalar2=None,
                    op0=mybir.AluOpType.mult,
                )
                for b_img in range(IPC):
                    nc.scalar.dma_start(
                        out[b_img * (NF + NCO) + NF:b_img * (NF + NCO) + NF + NCO, :],
                        oc[b_img * NCO:(b_img + 1) * NCO, :],
                    )

        # --- emit with 1-stage software pipelining: T(s+1) before M(s) ---
        stages = [("f", g) for g in range(GF)] + [("c", tt) for tt in range(NTC)]
        prev = None
        for kind, i in stages:
            lt = fine_T(i) if kind == "f" else coarse_T(i)
            if prev is not None:
                pk, pi, plt = prev
                (fine_M if pk == "f" else coarse_M)(pi, plt)
            prev = (kind, i, lt)
        pk, pi, plt = prev
        (fine_M if pk == "f" else coarse_M)(pi, plt)


def build(reps: int = 1):
    nc = bacc.Bacc("TRN2", target_bir_lowering=False, debug=False)
    t = {
        "imgs16": nc.dram_tensor("imgs16", [IPC * IMG_F, 1], BDT, kind="ExternalInput"),
        "imgs8c": nc.dram_tensor("imgs8c", [IPC * IMG_C, 1], F8, kind="ExternalInput"),
        "wf2": nc.dram_tensor("wf2", [P, (KF // P) * D], BDT, kind="ExternalInput"),
        "wc2": nc.dram_tensor("wc2", [P, (KC // P) * D], F8, kind="ExternalInput"),
        "bias_f": nc.dram_tensor("bias_f", [P, D], FDT, kind="ExternalInput"),
        "bias_c": nc.dram_tensor("bias_c", [P, D], FDT, kind="ExternalInput"),
        "ident": nc.dram_tensor("ident", [P, P], FDT, kind="ExternalInput"),
        "fidx": nc.dram_tensor("fidx", [P, GF], IDT, kind="ExternalInput"),
        "cidx": nc.dram_tensor("cidx", [P, 1], IDT, kind="ExternalInput"),
        "out": nc.dram_tensor("out", [IPC * (NF + NCO), D], BDT, kind="ExternalOutput"),
    }
    with tile.TileContext(nc) as tc:
        for _ in range(reps):
            _emit(nc, tc, t)
    nc.compile()
    return nc


def repack_fine(images):
    """[b, C, H, W] f32 -> sliding 16-row-block channel-last bf16 blob.

    blk[b, y, x, dy, c] = images[b, c, y+dy, x], y in [0, H-16]."""
    cl = np.ascontiguousarray(images.transpose(0, 2, 3, 1)).astype(BF16)
    sw = np.lib.stride_tricks.sliding_window_view(cl, FP, axis=1)  # [b,497,x,c,dy]
    return np.ascontiguousarray(sw.transpose(0, 1, 2, 4, 3))


def repack_coarse(images):
    """[b, C, H, W] f32 -> sliding 64-row-block channel-last e3m4 blob."""
    cl = np.ascontiguousarray(images.transpose(0, 2, 3, 1)).astype(E3M4)
    sw = np.lib.stride_tricks.sliding_window_view(cl, CP, axis=1)  # [b,449,x,c,dy]
    return np.ascontiguousarray(sw.transpose(0, 1, 2, 4, 3))


def host_indices(fine_xy, coarse_xy):
    """Element offsets into the per-core blobs (one per gather run)."""
    # fine: group g = (img b, half hh); partition p -> patch hh*128+p
    base_f = fine_xy[:, :, 1] * BLK_F + fine_xy[:, :, 0] * (FP * C) \
        + (np.arange(IPC) * IMG_F)[:, None]                        # [IPC, NF]
    fidx = base_f.reshape(GF, P).T                                 # [P, GF]
    base_c = coarse_xy[:, :, 1] * BLK_C + coarse_xy[:, :, 0] * (CP * C) \
        + (np.arange(IPC) * IMG_C)[:, None]                        # [IPC, NCO]
    cidx = base_c.reshape(P, 1)
    return (np.ascontiguousarray(fidx.astype(np.int32)),
            np.ascontiguousarray(cidx.astype(np.int32)))


def feat_perm(patch):
    """Gather order (dx, dy, c) -> original (c, dy, dx) column index."""
    dx, dy, c = np.meshgrid(
        np.arange(patch), np.arange(patch), np.arange(C), indexing="ij"
    )
    return (c * (patch * patch) + dy * patch + dx).reshape(-1)


def swizzle_w_interleave(wg, stride):
    """[K, D] gather-order weights -> [128, (K//128)*D], rows interleaved so
    block (j, b) holds rows (128*stride)*j + stride*i + b (i = partition)."""
    K = wg.shape[0]
    blocks = []
    for j in range(K // (P * stride)):
        for b in range(stride):
            blocks.append(wg[P * stride * j + stride * np.arange(P) + b])
    return np.ascontiguousarray(
        np.stack(blocks, axis=1).reshape(P, (K // P) * D)
    )


def make_in_maps(images, W_fine, b_fine, W_coarse, b_coarse, fine_xy, coarse_xy):
    images = np.asarray(images, dtype=np.float32)
    fine_xy = np.asarray(fine_xy, dtype=np.int64)
    coarse_xy = np.asarray(coarse_xy, dtype=np.int64)
    blob_f = repack_fine(images)
    blob_c = repack_coarse(images)
    wf2 = swizzle_w_interleave(
        np.asarray(W_fine, np.float32).T[feat_perm(FP)].astype(BF16), 2)
    wc2 = swizzle_w_interleave(
        (np.asarray(W_coarse, np.float32).T[feat_perm(CP)] * S_W).astype(E3M4), 4)
    bias_f = np.ascontiguousarray(
        np.repeat(np.asarray(b_fine, np.float32)[None, :], P, axis=0))
    bias_c = np.ascontiguousarray(
        np.repeat(np.asarray(b_coarse, np.float32)[None, :] * S_W, P, axis=0))
    ident = np.eye(P, dtype=np.float32)
    in_maps = []
    for c in range(NCORES):
        sl = slice(c * IPC, (c + 1) * IPC)
        fidx, cidx = host_indices(fine_xy[sl], coarse_xy[sl])
        in_maps.append({
            "imgs16": blob_f[sl].reshape(IPC * IMG_F, 1),
            "imgs8c": blob_c[sl].reshape(IPC * IMG_C, 1),
            "wf2": wf2, "wc2": wc2,
            "bias_f": bias_f, "bias_c": bias_c, "ident": ident,
            "fidx": fidx, "cidx": cidx,
        })
    return in_maps


_NC_CACHE = []


def _get_nc():
    if not _NC_CACHE:
        _NC_CACHE.append(build())
    return _NC_CACHE[0]


def run(inputs: dict, trace: bool = False):
    nc = _get_nc()
    in_maps = make_in_maps(**inputs)
    res = run_bass_kernel_spmd(nc, in_maps, list(range(NCORES)), trace=trace)
    outs = [
        np.asarray(res.results[c]["out"]).astype(np.float32).reshape(IPC, NF + NCO, D)
        for c in range(NCORES)
    ]
    return np.concatenate(outs, axis=0), res


def kernel(**inputs) -> np.ndarray:
    out, _ = run(inputs, trace=False)
    return out
